# revision 2
# baseline (speedup 1.0000x reference)
"""Blinn-Phong env-map shader on 8 Trainium2 NeuronCores.

Sharding: pixel rows (H) split 8 ways, one slice of 32x256=8192 pixels per
core; vertex/face tables and light tables replicated. No cross-core
communication.

Per-core pipeline:
  1. indirect-DMA gathers: faces[pix_to_face] then verts_normals[...]
  2. DVE barycentric interpolation + L2 normalize (ACT sqrt + DVE recip)
  3. DVE 32x32 block transposes -> pn^T replicated into 4 row-groups
  4. K=3 row-group-tiled matmuls -> diffuse scores in PSUM (j x pixel tiles)
  5. clip(0,1): DVE dual-op tensor_scalar, or ACT 2-pass relu for a static
     subset of j-tiles (r2 = relu(1-relu(x)); sign folded into weights and a
     constant matmul term)
  6. col-group-tiled K=128 matmuls accumulate colors^T in PSUM
  7. DVE multiply by texels, DMA out
Host side only shards/permutes layouts and unscrambles outputs.
"""
import sys
if "/opt/trn_rl_repo" not in sys.path:
    sys.path.insert(0, "/opt/trn_rl_repo")

import numpy as np
import concourse.bass as bass
import concourse.tile as tile
from concourse import bacc, mybir
from concourse.bass_utils import run_bass_kernel_spmd

F32 = mybir.dt.float32
I32 = mybir.dt.int32
OP = mybir.AluOpType
AF = mybir.ActivationFunctionType

V, F, H, W, LB, J = 50000, 100000, 256, 256, 2, 1024
NCORES = 8
ROWS = H // NCORES          # 32 pixel rows per core
PX = ROWS * W               # 8192 pixels per core
NCH = PX // 128             # 64 chunks of 128 pixels
NB = PX // 2048             # 4 pixel-block windows of 2048 px
NJT = (LB * J) // 128       # 16 j-tiles
EPS = 1e-6
# j-tiles whose clip runs on the scalar engine via the 2-pass relu trick
# (the rest run on the vector engine via dual-op tensor_scalar)
ACT_JT = (1, 4, 7, 10, 12, 15)

_CACHE = {}


def _pixmap():
    """p_dev[B, g, w] for the mm2 col-group pixel layout."""
    B = np.arange(NB)[:, None, None]
    g = np.arange(4)[None, :, None]
    w = np.arange(512)[None, None, :]
    return (16 * B + w // 32) * 128 + 32 * g + (w % 32)


def _build():
    nc = bacc.Bacc("TRN2", target_bir_lowering=False, debug=False)

    p2fg = nc.dram_tensor("p2fg", [128, NCH], I32, kind="ExternalInput")
    baryg = nc.dram_tensor("baryg", [128, NCH * 3], F32, kind="ExternalInput")
    texg = nc.dram_tensor("texg", [128, NB * 512], F32, kind="ExternalInput")
    ldT = nc.dram_tensor("ldT", [128, NJT * 128], F32, kind="ExternalInput")
    envT = nc.dram_tensor("envT", [128, NJT * 6], F32, kind="ExternalInput")
    facesd = nc.dram_tensor("faces", [F, 3], I32, kind="ExternalInput")
    vnormd = nc.dram_tensor("vnorm", [V, 3], F32, kind="ExternalInput")
    kdin = nc.dram_tensor("kdin", [1, 1], F32, kind="ExternalInput")

    out_colors = nc.dram_tensor("out_colors", [128, NB * 512], F32,
                                kind="ExternalOutput")
    out_pn = nc.dram_tensor("out_pn", [128, NCH * 3], F32,
                            kind="ExternalOutput")

    with tile.TileContext(nc) as tc:
        with tc.tile_pool(name="singles", bufs=1) as singles, \
             tc.tile_pool(name="work", bufs=1) as work, \
             tc.tile_pool(name="cdp", bufs=3) as cdp, \
             tc.tile_pool(name="r1p", bufs=2) as r1p, \
             tc.tile_pool(name="outp", bufs=2) as outp, \
             tc.tile_pool(name="dps", bufs=2, space="PSUM") as dps, \
             tc.tile_pool(name="cps", bufs=2, space="PSUM") as cps, \
             tc.tile_pool(name="smallps", bufs=1, space="PSUM") as smallps:

            # ---- constant loads
            ld_sb = singles.tile([128, NJT * 128], F32)
            nc.sync.dma_start(out=ld_sb[:], in_=ldT[:, :])
            envT_sb = singles.tile([128, NJT * 6], F32)
            nc.sync.dma_start(out=envT_sb[:], in_=envT[:, :])
            tex_sb = singles.tile([128, NB * 512], F32)
            nc.sync.dma_start(out=tex_sb[:], in_=texg[:, :])
            p2f_sb = singles.tile([128, NCH], I32)
            nc.sync.dma_start(out=p2f_sb[:], in_=p2fg[:, :])
            bary_sb = singles.tile([128, NCH * 3], F32)
            nc.sync.dma_start(out=bary_sb[:], in_=baryg[:, :])
            kd_sb = singles.tile([128, 1], F32)
            kd_bcast = bass.AP(tensor=kdin[:, :].tensor, offset=0,
                               ap=[[0, 128], [1, 1]])
            nc.gpsimd.dma_start(out=kd_sb[:], in_=kd_bcast)
            ones_sb = singles.tile([1, 512], F32)
            nc.vector.memset(ones_sb[:], 1.0)
            zeros_sb = singles.tile([1, 128], F32)
            nc.vector.memset(zeros_sb[:], 0.0)
            ones128_sb = singles.tile([128, 1], F32)
            nc.vector.memset(ones128_sb[:], 1.0)

            # ---- light-color weights: lc = env * clip(kd); neg variant for ACT j-tiles
            kdc = work.tile([128, 1], F32)
            nc.vector.tensor_scalar(out=kdc[:], in0=kd_sb[:], scalar1=0.0,
                                    scalar2=1.0, op0=OP.max, op1=OP.min)
            lcT_pos = singles.tile([128, NJT * 6], F32)
            nc.vector.tensor_scalar(out=lcT_pos[:], in0=envT_sb[:],
                                    scalar1=kdc[:, 0:1], scalar2=None,
                                    op0=OP.mult)
            lcT_neg = singles.tile([128, NJT * 6], F32)
            nc.vector.tensor_scalar(out=lcT_neg[:], in0=lcT_pos[:],
                                    scalar1=-1.0, scalar2=None, op0=OP.mult)

            # C[s] = sum over ACT j-tiles of lc_pos[j, s]
            c_ps = smallps.tile([1, 6], F32)
            for i, jt in enumerate(ACT_JT):
                nc.tensor.matmul(out=c_ps[0:1, 0:6],
                                 lhsT=ones128_sb[:, 0:1],
                                 rhs=lcT_pos[:, jt * 6:(jt + 1) * 6],
                                 start=(i == 0), stop=(i == len(ACT_JT) - 1))
            c_sb = work.tile([1, 6], F32)
            nc.vector.tensor_copy(out=c_sb[:], in_=c_ps[:])
            cfull_sb = singles.tile([1, 128], F32)
            nc.vector.memset(cfull_sb[:], 0.0)
            cfull_v = cfull_sb[:].rearrange("p (g s) -> p g s", s=32)
            for g in range(4):
                nc.vector.tensor_copy(out=cfull_v[:, g, 0:6], in_=c_sb[:, :])

            # ---- gathers: HW indirect DMA consumes ONE index per partition
            # per instruction (dest = one contiguous run per partition), so
            # loop over chunk columns.
            fv_sb = work.tile([128, NCH * 3], I32)
            for n in range(NCH):
                nc.gpsimd.indirect_dma_start(
                    out=fv_sb[:, 3 * n:3 * n + 3], out_offset=None,
                    in_=facesd[:, :],
                    in_offset=bass.IndirectOffsetOnAxis(
                        ap=p2f_sb[:, n:n + 1], axis=0))
            vn_sb = work.tile([128, NCH * 9], F32)
            for n in range(NCH):
                for k in range(3):
                    nc.gpsimd.indirect_dma_start(
                        out=vn_sb[:, 9 * n + 3 * k:9 * n + 3 * k + 3],
                        out_offset=None, in_=vnormd[:, :],
                        in_offset=bass.IndirectOffsetOnAxis(
                            ap=fv_sb[:, 3 * n + k:3 * n + k + 1], axis=0))

            # ---- barycentric interpolation + normalize
            brep_sb = work.tile([128, NCH * 9], F32)
            nc.vector.tensor_copy(
                out=brep_sb[:],
                in_=bary_sb[:].rearrange("p (n k) -> p n k", k=3)
                    .to_broadcast([128, NCH, 3, 3]))
            prod = work.tile([128, NCH * 9], F32)
            nc.vector.tensor_tensor(out=prod[:], in0=vn_sb[:], in1=brep_sb[:],
                                    op=OP.mult)
            prod_v = prod[:].rearrange("p (n k c) -> p n k c", k=3, c=3)
            pn_sb = work.tile([128, NCH * 3], F32)
            pn_v = pn_sb[:].rearrange("p (n c) -> p n c", c=3)
            nc.vector.tensor_tensor(out=pn_v, in0=prod_v[:, :, 0, :],
                                    in1=prod_v[:, :, 1, :], op=OP.add)
            nc.vector.tensor_tensor(out=pn_v, in0=pn_v,
                                    in1=prod_v[:, :, 2, :], op=OP.add)
            sq = work.tile([128, NCH * 3], F32)
            nc.vector.tensor_tensor(out=sq[:], in0=pn_sb[:], in1=pn_sb[:],
                                    op=OP.mult)
            sq_v = sq[:].rearrange("p (n c) -> p n c", c=3)
            ss = work.tile([128, NCH], F32)
            nc.vector.tensor_tensor(out=ss[:], in0=sq_v[:, :, 0],
                                    in1=sq_v[:, :, 1], op=OP.add)
            nc.vector.tensor_tensor(out=ss[:], in0=ss[:], in1=sq_v[:, :, 2],
                                    op=OP.add)
            nrm = work.tile([128, NCH], F32)
            nc.scalar.activation(out=nrm[:], in_=ss[:], func=AF.Sqrt)
            nc.vector.tensor_scalar(out=nrm[:], in0=nrm[:], scalar1=EPS,
                                    scalar2=None, op0=OP.max)
            inv = work.tile([128, NCH], F32)
            nc.vector.reciprocal(out=inv[:], in_=nrm[:])
            pnh_sb = work.tile([128, NCH * 3], F32)
            nc.vector.tensor_tensor(out=pnh_sb[:].rearrange(
                                        "p (n c) -> p n c", c=3),
                                    in0=pn_v,
                                    in1=inv[:].to_broadcast([128, NCH, 3]),
                                    op=OP.mult)
            nc.sync.dma_start(out=out_pn[:, :], in_=pnh_sb[:])

            # ---- padded layout + 32x32 block transposes
            pnpad_sb = singles.tile([128, NCH * 32], F32)
            nc.vector.memset(pnpad_sb[:], 0.0)
            nc.vector.tensor_copy(
                out=pnpad_sb[:].rearrange("p (n z) -> p n z", z=32)[:, :, 0:3],
                in_=pnh_sb[:].rearrange("p (n c) -> p n c", c=3))
            pnT_sb = singles.tile([128, NB * 512], F32)
            for T in range(NB):
                nc.vector.transpose(out=pnT_sb[:, T * 512:(T + 1) * 512],
                                    in_=pnpad_sb[:, T * 512:(T + 1) * 512])

            # ---- main loop over pixel-block windows
            for B in range(NB):
                colors_ps = cps.tile([128, 512], F32)
                nc.tensor.matmul(out=colors_ps[:, :],
                                 lhsT=zeros_sb[0:1, 0:128],
                                 rhs=ones_sb[0:1, 0:512],
                                 start=True, stop=False, tile_position=(0, 0),
                                 skip_group_check=True)
                for jt in range(NJT):
                    use_act = jt in ACT_JT
                    for h in range(2):
                        d_ps = dps.tile([128, 1024], F32, tag="d")
                        for ri in range(2):
                            r = 2 * h + ri
                            nc.tensor.matmul(
                                out=d_ps[:, ri * 512:(ri + 1) * 512],
                                lhsT=ld_sb[32 * r:32 * r + 3,
                                           jt * 128:(jt + 1) * 128],
                                rhs=pnT_sb[32 * r:32 * r + 3,
                                           B * 512:(B + 1) * 512],
                                start=True, stop=True,
                                tile_position=(32 * r, 0))
                        cd = cdp.tile([128, 1024], F32, tag="cd")
                        if use_act:
                            r1 = r1p.tile([128, 1024], F32, tag="r1")
                            nc.scalar.activation(out=r1[:], in_=d_ps[:],
                                                 func=AF.Relu)
                            nc.scalar.activation(out=cd[:], in_=r1[:],
                                                 func=AF.Relu, bias=1.0,
                                                 scale=-1.0)
                            wsel = lcT_neg
                        else:
                            nc.vector.tensor_scalar(out=cd[:], in0=d_ps[:],
                                                    scalar1=0.0, scalar2=1.0,
                                                    op0=OP.max, op1=OP.min)
                            wsel = lcT_pos
                        for ri in range(2):
                            g = 2 * h + ri
                            nc.tensor.matmul(
                                out=colors_ps[32 * g:32 * g + 6, :],
                                lhsT=wsel[:, jt * 6:jt * 6 + 6],
                                rhs=cd[:, ri * 512:(ri + 1) * 512],
                                start=False, stop=False,
                                tile_position=(0, 32 * g),
                                skip_group_check=True)
                nc.tensor.matmul(out=colors_ps[:, :],
                                 lhsT=cfull_sb[0:1, 0:128],
                                 rhs=ones_sb[0:1, 0:512],
                                 start=False, stop=True,
                                 tile_position=(0, 0), skip_group_check=True)
                out1 = outp.tile([128, 512], F32, tag="out1")
                nc.vector.tensor_tensor(out=out1[:], in0=colors_ps[:],
                                        in1=tex_sb[:, B * 512:(B + 1) * 512],
                                        op=OP.mult)
                nc.sync.dma_start(out=out_colors[:, B * 512:(B + 1) * 512],
                                  in_=out1[:])

    nc.compile()
    return nc


def _prep_in_maps(inputs):
    verts_normals = np.ascontiguousarray(inputs["verts_normals"],
                                         dtype=np.float32)
    faces = np.ascontiguousarray(inputs["faces"], dtype=np.int32)
    pix_to_face = np.asarray(inputs["pix_to_face"], dtype=np.int32)
    bary_coords = np.asarray(inputs["bary_coords"], dtype=np.float32)
    light_dirs = np.asarray(inputs["light_dirs"], dtype=np.float32)
    env_map = np.asarray(inputs["env_map"], dtype=np.float32)
    texels = np.asarray(inputs["texels"], dtype=np.float32)
    kd = np.asarray(inputs["kd"], dtype=np.float32)

    ldflat = light_dirs.reshape(LB * J, 3)
    ldT = np.zeros((128, NJT * 128), np.float32)
    for r in range(4):
        ldT[32 * r:32 * r + 3, :] = ldflat.T
    envflat = env_map.reshape(LB * J, 3)
    jj = np.arange(LB * J)
    E = np.zeros((128, NJT, 6), np.float32)
    for c in range(3):
        E[jj % 128, jj // 128, 3 * (jj // J) + c] = envflat[:, c]
    envT = E.reshape(128, NJT * 6)
    kdin = kd.reshape(1, 1)
    pm = _pixmap()  # [NB, 4, 512]

    in_maps = []
    for core in range(NCORES):
        rows = slice(core * ROWS, (core + 1) * ROWS)
        p2f_core = pix_to_face[0, rows, :, 0].reshape(PX)
        bary_core = bary_coords[0, rows, :, 0, :].reshape(PX, 3)
        tex_core = texels[:, rows, :, :].reshape(LB, PX, 3)

        p2fg = np.ascontiguousarray(p2f_core.reshape(NCH, 128).T)
        baryg = np.ascontiguousarray(
            bary_core.reshape(NCH, 128, 3).transpose(1, 0, 2)
            .reshape(128, NCH * 3))
        t = tex_core[:, pm, :]                      # [2, NB, 4, 512, 3]
        X = np.zeros((4, 32, NB, 512), np.float32)
        X[:, :6] = t.transpose(2, 0, 4, 1, 3).reshape(4, 6, NB, 512)
        texg = np.ascontiguousarray(X.reshape(128, NB * 512))

        in_maps.append({
            "p2fg": p2fg, "baryg": baryg, "texg": texg,
            "ldT": ldT, "envT": envT, "faces": faces,
            "vnorm": verts_normals, "kdin": kdin,
        })
    return in_maps


def _postprocess(results):
    pm = _pixmap()
    colors = np.empty((LB, H, W, 3), np.float32)
    pn = np.empty((H, W, 3), np.float32)
    for core in range(NCORES):
        oc = np.asarray(results[core]["out_colors"])  # [128, NB*512]
        op_ = np.asarray(results[core]["out_pn"])     # [128, NCH*3]
        Y = oc.reshape(4, 32, NB, 512)
        ccore = np.empty((LB, PX, 3), np.float32)
        for b in range(LB):
            for c in range(3):
                ccore[b, pm, c] = Y[:, 3 * b + c].transpose(1, 0, 2)
        rows = slice(core * ROWS, (core + 1) * ROWS)
        colors[:, rows] = ccore.reshape(LB, ROWS, W, 3)
        pn[rows] = (op_.reshape(128, NCH, 3).transpose(1, 0, 2)
                    .reshape(PX, 3).reshape(ROWS, W, 3))
    pn_full = np.broadcast_to(pn[None], (LB, H, W, 3)).copy()
    return colors, pn_full


def kernel(**inputs):
    if "nc" not in _CACHE:
        _CACHE["nc"] = _build()
    nc = _CACHE["nc"]
    in_maps = _prep_in_maps(inputs)
    res = run_bass_kernel_spmd(nc, in_maps, list(range(NCORES)))
    return _postprocess(res.results)
